# revision 19
# baseline (speedup 1.0000x reference)
"""Trainium2 Bass kernel for nn_PlasticityModelMoE (8-core SPMD).

Strategy (v2, fp8):
  Host precomputes all weight transforms: wmod = w*sigmoid(delay)*conn*mask*64
  (b-major, fp8e4m3) concatenated with gate_W*64; x pre-transposed to fp8 xT;
  read_W shard scaled *64 fp8; memory shard (+ones col) fp16; connectivity MLP
  and activation-blend polynomial coefs (scaled *16) computed on host.
  Device: phase 1 (units tensor-parallel, 256/core): branch+gate logits via
  fp8 DoubleRow matmuls (K=256/step), gate softmax, z-combine, relu, degree-4
  Horner blend -> fp8 blendT (*16).  Per 512-col batch chunk: fp8 AllGather of
  blendT, then phase 3 (memory-rows tensor-parallel, 1024/core): logitsT =
  rw8 x bT via fp8 DoubleRow, exp (descale 2^-10, +read_b) -> fp16 expT.
  Phase 4: [read_partial | s] = E @ [mem | 1] in fp16, fp16 ReduceScatter over
  batch rows, divide by s -> each core emits its 256-row f32 output shard.
  A dummy 64B AllGather at t=0 absorbs the first-collective rendezvous.
"""
import numpy as np
from contextlib import ExitStack

import concourse.bass as bass
import concourse.mybir as mybir
import concourse.tile as tile
from concourse import bacc
from concourse.bass_utils import run_bass_kernel_spmd
from concourse.masks import make_identity

F32 = mybir.dt.float32
BF16 = mybir.dt.bfloat16
F16 = mybir.dt.float16
F8 = mybir.dt.float8e4
AF = mybir.ActivationFunctionType
ALU = mybir.AluOpType
AX = mybir.AxisListType
PM = mybir.MatmulPerfMode

KC = 8
N, D, U, NB, M, MD = 2048, 1024, 2048, 4, 8192, 1024
US = U // KC          # 256 units per core
MS = M // KC          # 1024 memory rows per core
NS = N // KC          # 256 output rows per core
NT = N // 128         # 16 batch tiles
DK = D // 128         # 8 k-tiles over D
DR = DK // 2          # 4 DoubleRow steps over D
UK = U // 128         # 16 k-tiles over U
UR = UK // 2          # 8 DoubleRow steps over U
MK = MS // 128        # 8 k-tiles over memory shard
UBF = US * NB         # 1024 branch columns per core
TS = [2, 4, 4, 4, 2]  # tiles (128 rows) per collective chunk
TOFF = [0, 2, 6, 10, 14]
NCH = len(TS)
SC_W = 64.0           # fp8 weight scale (2^6)
SC_B = 16.0           # fp8 blend scale (2^4)
DESC = 1.0 / (SC_W * SC_B)

_CMAT = np.array([
    [5.0000238e-01, 2.4987496e-01, 1.0582031e-03, -2.4046743e-02, 4.1678566e-03],
    [0.0, 1.0, 0.0, 0.0, 0.0],
    [-7.2632770e-06, 9.9976927e-01, 9.2018498e-03, -3.9401752e-01, 1.4669961e-01],
    [0.0, 1.0, 0.0, 0.0, 0.0],
    [8.6798245e-06, 4.9957812e-01, 2.5321743e-01, -8.1970906e-03, -1.3558048e-02],
    [3.9388153e-05, 4.9807969e-01, 4.1364601e-01, -3.7666172e-02, -3.2796454e-02],
    [0.0, 1.0507009873554805, 0.0, 0.0, 0.0],
    [3.1482985e-05, 5.9846270e-01, 3.3178753e-01, -4.6201140e-02, -1.9015398e-02],
    [0.0, 0.0, 0.0, 0.0, 0.0],
], dtype=np.float64)

_cache = {}


def _build(has_bias):
    nc = bacc.Bacc(num_devices=KC)

    xt_d = nc.dram_tensor("xt", [D, N], F8, kind="ExternalInput")
    wd_d = nc.dram_tensor("wd", [D, UBF + NB], F8, kind="ExternalInput")
    bias_d = nc.dram_tensor("bias", [UBF + NB], BF16, kind="ExternalInput")
    coefs_d = nc.dram_tensor("coefs", [128, 5], F32, kind="ExternalInput")
    rw_d = nc.dram_tensor("rw", [U, MS], F8, kind="ExternalInput")
    rb_d = nc.dram_tensor("rb", [MS], F32, kind="ExternalInput")
    mem_d = nc.dram_tensor("mem", [MS, MD + 2], F8, kind="ExternalInput")
    colsum_d = nc.dram_tensor("colsum", [MD + 1], F16, kind="ExternalInput")
    y_d = nc.dram_tensor("y", [NS, MD + 1], F16, kind="ExternalOutput")

    with tile.TileContext(nc) as tc, ExitStack() as ctx:
        consts = ctx.enter_context(tc.tile_pool(name="consts", bufs=1))
        big = ctx.enter_context(tc.tile_pool(name="big", bufs=1))
        p3p = ctx.enter_context(tc.tile_pool(name="p3p", bufs=2))
        p4p = ctx.enter_context(tc.tile_pool(name="p4p", bufs=2))
        blendp = ctx.enter_context(tc.tile_pool(name="blendp", bufs=2))
        dram_ag = ctx.enter_context(tc.tile_pool(name="dram_ag", bufs=1, space="DRAM"))
        dram_rs = ctx.enter_context(tc.tile_pool(name="dram_rs", bufs=1, space="DRAM"))
        # PSUM budget (8 banks): br [128,1028] f32 ~2 banks x 2 bufs,
        # tr [128,<=512] 1 bank x 2 bufs
        psum = ctx.enter_context(tc.tile_pool(name="psum", bufs=2, space="PSUM"))

        # ---------- tiny consts ----------
        idf = consts.tile([128, 128], F32)
        make_identity(nc, idf)
        idb = consts.tile([128, 128], BF16)
        nc.any.tensor_copy(idb, idf)
        ones_lhs = consts.tile([1, 128], BF16)
        nc.vector.memset(ones_lhs, 1.0)
        ones16 = consts.tile([1, 128], F16)
        nc.vector.memset(ones16, 1.0)
        colsum_sb = consts.tile([1, MD + 1], F16)
        nc.sync.dma_start(out=colsum_sb, in_=colsum_d.ap()[None])
        bias_b = consts.tile([1, UBF + NB], BF16)
        nc.sync.dma_start(out=bias_b, in_=bias_d.ap()[None])
        coefs = consts.tile([128, 5], F32)
        nc.sync.dma_start(out=coefs, in_=coefs_d[:, :])
        rb_sb = consts.tile([128, MK], F32)
        nc.sync.dma_start(out=rb_sb, in_=rb_d.ap().rearrange("(t p) -> p t", p=128))

        # ---------- big input loads (sync queue, priority order) ----------
        wm = big.tile([128, DK, UBF + NB], F8)
        nc.sync.dma_start(out=wm, in_=wd_d.ap().rearrange("(t p) c -> p t c", p=128))
        xT = big.tile([128, DK, N], F8)
        for c in range(4):
            csl = slice(c * 512, (c + 1) * 512)
            nc.sync.dma_start(
                out=xT[:, :, csl],
                in_=xt_d.ap()[:, csl].rearrange("(t p) n -> p t n", p=128))
        rw8 = big.tile([128, UK, MS], F8)
        nc.sync.dma_start(out=rw8, in_=rw_d.ap().rearrange("(t p) m -> p t m", p=128))
        mem8 = big.tile([128, MK, MD + 2], F8)
        nc.sync.dma_start(out=mem8, in_=mem_d.ap().rearrange("(t p) c -> p t c", p=128))

        blendT = big.tile([128, 2, N], F8)
        ag_outs = []
        expTs = []
        rs_outs = []
        blend16s = {}

        def emit_tile(i):
            nsl = slice(i * 128, (i + 1) * 128)
            br = psum.tile([128, UBF], F32, tag="br", name="br")
            gt_ps = psum.tile([128, NB], F32, tag="sm", name="gt_ps")
            for (c0, c1) in [(0, 512), (512, 1024), (1024, 1028)]:
                out = br[:, c0:c1] if c1 <= UBF else gt_ps
                for s in range(DR):
                    nc.tensor.matmul(out,
                                     xT[:, 2 * s:2 * s + 2, nsl],
                                     wm[:, 2 * s:2 * s + 2, c0:c1],
                                     start=(s == 0),
                                     stop=(not has_bias and s == DR - 1),
                                     perf_mode=PM.DoubleRow)
                if has_bias:
                    nc.tensor.matmul(out, ones_lhs, bias_b[:, c0:c1],
                                     start=False, stop=True,
                                     skip_group_check=True)
            # gate softmax on br[:, 1024:1028] (logits are *SC_W; exp safe
            # without max-sub: true |logit| <~ 4)
            g_exp = blendp.tile([128, NB], F32, tag="g1")
            nc.scalar.activation(g_exp, gt_ps, AF.Exp, scale=1.0 / SC_W)
            g_sum = blendp.tile([128, 1], F32, tag="g2")
            nc.vector.tensor_reduce(g_sum, g_exp, AX.X, ALU.add)
            g_rec = blendp.tile([128, 1], F32, tag="g3")
            nc.vector.reciprocal(g_rec, g_sum)
            gate = blendp.tile([128, NB], F32, tag="g4")
            nc.vector.tensor_scalar(gate, g_exp, g_rec[:, 0:1], 1.0 / SC_W,
                                    ALU.mult, ALU.mult)
            # z = sum_b gate_b * branch_b  (bf16 pipeline)
            zt0 = blendp.tile([128, US], BF16, tag="t0")
            nc.scalar.activation(zt0, br[:, 0:US], AF.Copy,
                                 scale=gate[:, 0:1])
            zt1 = blendp.tile([128, US], BF16, tag="t1")
            nc.scalar.activation(zt1, br[:, US:2 * US], AF.Copy,
                                 scale=gate[:, 1:2])
            zt2 = blendp.tile([128, US], BF16, tag="t2")
            nc.scalar.activation(zt2, br[:, 2 * US:3 * US], AF.Copy,
                                 scale=gate[:, 2:3])
            zt3 = blendp.tile([128, US], BF16, tag="t3")
            nc.scalar.activation(zt3, br[:, 3 * US:4 * US], AF.Copy,
                                 scale=gate[:, 3:4])
            z01 = blendp.tile([128, US], BF16, tag="t0")
            nc.vector.tensor_add(z01, zt0, zt1)
            z23 = blendp.tile([128, US], BF16, tag="t2")
            nc.vector.tensor_add(z23, zt2, zt3)
            z_sb = blendp.tile([128, US], BF16, tag="t1")
            nc.vector.tensor_add(z_sb, z01, z23)
            a_sb = blendp.tile([128, US], BF16, tag="ta")
            nc.vector.tensor_scalar_max(a_sb, z_sb, 0.0)
            # blend*16 = c0 + c1 a + c2 a^2 + c3 a^3 + c4 a^4 via even/odd
            # split on a^2 (coefs prescaled *16, shorter dep chain)
            a2 = blendp.tile([128, US], BF16, tag="t0")
            nc.vector.tensor_mul(a2, a_sb, a_sb)
            te = blendp.tile([128, US], BF16, tag="t1")
            nc.vector.tensor_scalar(te, a2, coefs[:, 4:5], coefs[:, 2:3],
                                    ALU.mult, ALU.add)
            to = blendp.tile([128, US], BF16, tag="t2")
            nc.vector.tensor_scalar(to, a2, coefs[:, 3:4], coefs[:, 1:2],
                                    ALU.mult, ALU.add)
            ev = blendp.tile([128, US], BF16, tag="t3")
            nc.vector.tensor_mul(ev, te, a2)
            od = blendp.tile([128, US], BF16, tag="t0")
            nc.vector.tensor_mul(od, to, a_sb)
            eo = blendp.tile([128, US], BF16, tag="t1")
            nc.vector.tensor_add(eo, ev, od)
            blend16 = blendp.tile([128, US], BF16, tag="bb", bufs=5)
            nc.vector.tensor_scalar_add(blend16, eo, coefs[:, 0:1])
            blend16s[i] = blend16

        def emit_transpose(i):
            nsl = slice(i * 128, (i + 1) * 128)
            blend16 = blend16s.pop(i)
            for uh in range(2):
                trb = psum.tile([128, 128], BF16, tag="sm", name="trb")
                nc.tensor.transpose(trb, blend16[:, uh * 128:(uh + 1) * 128], idb)
                nc.scalar.activation(blendT[:, uh, nsl], trb, AF.Copy)

        def emit_ag(ch):
            w = TS[ch] * 128
            csl = slice(TOFF[ch] * 128, TOFF[ch] * 128 + w)
            agi = dram_ag.tile([US, w], F8, name=f"ag_in{ch}", tag=f"agi{ch}")
            for uh in range(2):
                nc.gpsimd.dma_start(out=agi[uh * 128:(uh + 1) * 128, :],
                                    in_=blendT[:, uh, csl])
            ago = dram_ag.tile([U, w], F8, name=f"ag_out{ch}", tag=f"ago{ch}",
                               addr_space="Shared")
            nc.gpsimd.collective_compute(
                "AllGather", ALU.bypass, replica_groups=[list(range(KC))],
                ins=[agi.opt()], outs=[ago.opt()])
            ag_outs.append(ago)

        def emit_phase3(ch):
            w = TS[ch] * 128
            bT = p3p.tile([128, UK, w], F8, tag="bT", name="bT")
            for t in range(UK):
                nc.sync.dma_start(out=bT[:, t, :],
                                  in_=ag_outs[ch][t * 128:(t + 1) * 128, :])
            expT = p3p.tile([128, MK, w], F8, tag="expT", name="expT")
            for mk in range(MK):
                l_ps = psum.tile([128, w], F32, tag="lp", name="l_ps")
                for s in range(UR):
                    nc.tensor.matmul(l_ps,
                                     rw8[:, 2 * s:2 * s + 2,
                                         mk * 128:(mk + 1) * 128],
                                     bT[:, 2 * s:2 * s + 2, :],
                                     start=(s == 0), stop=(s == UR - 1),
                                     perf_mode=PM.DoubleRow)
                exp_sb = p3p.tile([128, w], F32, tag="exps", name="exp_sb")
                nc.scalar.activation(exp_sb, l_ps, AF.Exp,
                                     bias=rb_sb[:, mk:mk + 1], scale=DESC)
                # e8 = 8*(E-1): fp8-centered so mem can be fp8 too
                nc.vector.tensor_scalar(expT[:, mk, :], exp_sb, 8.0, -8.0,
                                        ALU.mult, ALU.add)
            expTs.append(expT)

        def emit_phase4(ch):
            w = TS[ch] * 128
            expT = expTs[ch]
            rs_inj = dram_rs.tile([w, MD + 1], F16, name=f"rs_in{ch}",
                                  tag=f"rsi{ch}")
            for sj in range(TS[ch]):
                jsl = slice(sj * 128, (sj + 1) * 128)
                r_ps = psum.tile([128, UBF], F32, tag="br", name="r_ps")
                s_ps = psum.tile([128, 1], F32, tag="sm", name="s_ps")
                for (c0, c1) in [(0, 512), (512, 1024), (1024, 1025)]:
                    out = r_ps[:, c0:c1] if c1 <= UBF else s_ps
                    nc.tensor.matmul(out, ones16, colsum_sb[:, c0:c1],
                                     start=True, stop=False,
                                     skip_group_check=True)
                    for s4 in range(MK // 2):
                        nc.tensor.matmul(out,
                                         expT[:, 2 * s4:2 * s4 + 2, jsl],
                                         mem8[:, 2 * s4:2 * s4 + 2, c0:c1],
                                         start=False, stop=(s4 == MK // 2 - 1),
                                         perf_mode=PM.DoubleRow,
                                         skip_group_check=True)
                r_sb = p4p.tile([128, MD + 1], F16, tag="rsb")
                nc.vector.tensor_scalar_mul(r_sb[:, 0:MD], r_ps, 0.125)
                nc.vector.tensor_scalar_mul(r_sb[:, MD:MD + 1], s_ps, 0.125)
                nc.gpsimd.dma_start(out=rs_inj[sj * 128:(sj + 1) * 128, :],
                                    in_=r_sb)
            yoff = TOFF[ch] * 128 // KC
            rs_out = dram_rs.tile([w // KC, MD + 1], F16, name=f"rs_out{ch}",
                                  tag=f"rso{ch}")
            nc.gpsimd.collective_compute(
                "ReduceScatter", ALU.add, replica_groups=[list(range(KC))],
                ins=[rs_inj.opt()], outs=[rs_out.opt()])
            nc.gpsimd.dma_start(out=y_d[yoff:yoff + w // KC, :],
                                in_=rs_out[:, :])

        # ---------- pipelined emission ----------
        for ch in range(NCH):
            for it in range(TS[ch]):
                emit_tile(TOFF[ch] + it)
            for it in range(TS[ch]):
                emit_transpose(TOFF[ch] + it)
            emit_ag(ch)
        for ch in range(NCH):
            emit_phase3(ch)
            emit_phase4(ch)

    nc.compile()
    return nc


def _sigmoid(v):
    return 1.0 / (1.0 + np.exp(-v))


def _make_in_maps(inputs):
    F8NP = mybir.dt.np(F8)
    x = np.asarray(inputs["x"], np.float32)
    w = np.asarray(inputs["w"], np.float64)
    delay = np.asarray(inputs["delay"], np.float64)
    b = np.asarray(inputs["b"], np.float64)
    gate_W = np.asarray(inputs["gate_W"], np.float64)
    gate_b = np.asarray(inputs["gate_b"], np.float64)
    na = np.asarray(inputs["neuron_avg"], np.float64)
    cw1 = np.asarray(inputs["conn_W1"], np.float64)
    cb1 = np.asarray(inputs["conn_b1"], np.float64)
    cw2 = np.asarray(inputs["conn_W2"], np.float64)
    cb2 = np.asarray(inputs["conn_b2"], np.float64)
    mask = np.asarray(inputs["mask"], np.float64)
    actw = np.asarray(inputs["act_w"], np.float64)
    read_W = np.asarray(inputs["read_W"], np.float32)
    read_b = np.asarray(inputs["read_b"], np.float32)
    mem = np.asarray(inputs["memory"], np.float32)

    # connectivity MLP (batch-independent, one row)
    h = np.maximum(na[None, :] @ cw1 + cb1, 0.0)
    conn = (_sigmoid(h @ cw2 + cb2)[0] * mask)            # [U]
    # activation-blend polynomial coefs, prescaled for fp8 blend
    e = np.exp(actw - actw.max())
    wts = e / e.sum()
    coefs = (wts @ _CMAT) * SC_B                          # [5]
    coefs_bc = np.ascontiguousarray(
        np.broadcast_to(coefs.astype(np.float32), (128, 5)))

    xt8 = np.ascontiguousarray(x.T).astype(F8NP)          # [D, N]
    wmod = w * _sigmoid(delay)                            # [D, U, NB]
    wmod = wmod * conn[None, :, None] * SC_W

    in_maps = []
    for k in range(KC):
        us, ue = k * US, (k + 1) * US
        ms, me = k * MS, (k + 1) * MS
        wd8 = np.concatenate(
            [wmod[:, us:ue, :].transpose(0, 2, 1).reshape(D, UBF),
             gate_W * SC_W], axis=1).astype(F8NP)
        bias_row = np.concatenate(
            [(b[us:ue] * conn[us:ue, None]).T.reshape(-1), gate_b]) * SC_W
        mem_aug = np.concatenate(
            [mem[ms:me], np.ones((MS, 1), np.float32),
             np.zeros((MS, 1), np.float32)], axis=1)
        colsum_row = np.concatenate(
            [mem[ms:me].astype(np.float64).sum(axis=0), [float(MS)]]) * 8.0
        in_maps.append({
            "xt": xt8,
            "wd": np.ascontiguousarray(wd8),
            "bias": bias_row.astype(mybir.dt.np(BF16)),
            "coefs": coefs_bc,
            "rw": np.ascontiguousarray(
                (read_W[:, ms:me] * np.float32(SC_W)).astype(F8NP)),
            "rb": np.ascontiguousarray(read_b[ms:me]),
            "mem": np.ascontiguousarray(mem_aug.astype(F8NP)),
            "colsum": colsum_row.astype(np.float16),
        })
    return in_maps


def kernel(**inputs) -> np.ndarray:
    in_maps = _make_in_maps(inputs)
    has_bias = any(np.any(m["bias"] != 0) for m in in_maps)
    key = ("nc", has_bias)
    if key not in _cache:
        _cache[key] = _build(has_bias)
        _cache["nc"] = _cache[key]
    nc = _cache[key]
    res = run_bass_kernel_spmd(nc, in_maps, core_ids=list(range(KC)))
    out = np.empty((N, MD), np.float32)
    for k in range(KC):
        yk = np.asarray(res.results[k]["y"], np.float32)
        yv = yk[:, 0:MD] / yk[:, MD:MD + 1]
        for ch in range(NCH):
            rows = TS[ch] * 128 // KC
            yoff = TOFF[ch] * 128 // KC
            dst = TOFF[ch] * 128 + k * rows
            out[dst:dst + rows] = yv[yoff:yoff + rows]
    return out
